# revision 1
# baseline (speedup 1.0000x reference)
"""Trainium2 Bass kernel for 16-head causal MultiHeadAttention.

Problem shapes (hardcoded): x [4, 2048, 1024], Wq/Wk/Wv/Wo [1024, 1024],
bo [1024]. 16 heads, head_dim 64, causal, softmax scale 1/8.

Sharding: tensor-parallel over heads. Core c owns heads {2c, 2c+1}, i.e.
feature slice [128c : 128c+128] of the QKV projections and the matching
input rows of the output projection. Each core computes q/k/v projections
for its slice over the whole (b, s) range, causal attention for its 8
(batch, head) pairs, and a partial out-projection [1024, 8192]^T. The
all-reduce over cores (and the bias add) is done host-side on the 8
partials.

Device layouts keep features on partitions:
  qT/kT/vT [128 (2 heads x 64), seq], scoresT [k, q] (softmax reduction
  over partitions via a ones-column appended to V in the ctx matmul),
  ctxT [128, seq], out_partial^T [1024 -> (128, 8), seq].

Matmul inputs are bf16 (PE at 1 cycle/row + fast weight load); PSUM
accumulation, softmax denominators, and the output path stay fp32.
The two heads are interleaved inside the attention loop so the PE always
has an independent stream while ACT computes the other head's exp.
"""

import numpy as np

B, S, D, H = 4, 2048, 1024, 16
HD = D // H  # 64
N_CORES = 8
ROWS = B * S  # 8192
RC = 512  # row-chunk (moving free dim)
QC = 512  # query chunk
KC = 128  # key chunk

_cache = {}


def _build():
    import concourse.bacc as bacc
    import concourse.tile as tile
    from concourse import mybir

    fp32 = mybir.dt.float32
    bf16 = mybir.dt.bfloat16

    nc = bacc.Bacc("TRN2", target_bir_lowering=False)

    xt_d = nc.dram_tensor("xt", [128, 8, ROWS], bf16, kind="ExternalInput")
    wq_d = nc.dram_tensor("wqt", [128, 8, 128], bf16, kind="ExternalInput")
    wk_d = nc.dram_tensor("wkt", [128, 8, 128], bf16, kind="ExternalInput")
    wv_d = nc.dram_tensor("wvt", [128, 8, 128], bf16, kind="ExternalInput")
    wo_d = nc.dram_tensor("wot", [128, 8, 128], bf16, kind="ExternalInput")
    mask_d = nc.dram_tensor("masks", [128, 4, QC], bf16, kind="ExternalInput")
    id_d = nc.dram_tensor("ident", [128, 128], bf16, kind="ExternalInput")
    out_d = nc.dram_tensor("outp", [128, 8, ROWS], bf16, kind="ExternalOutput")

    n_rc = S // RC  # row chunks per batch
    n_qc = S // QC  # query chunks per batch
    n_kc = S // KC  # key chunks per batch

    with tile.TileContext(nc) as tc:
        with (
            tc.tile_pool(name="const", bufs=1) as const_pool,
            tc.tile_pool(name="xt", bufs=3) as xt_pool,
            tc.tile_pool(name="proj", bufs=2) as proj_pool,
            tc.tile_pool(name="vaug", bufs=2) as vaug_pool,
            tc.tile_pool(name="attn", bufs=6) as attn_pool,
            tc.tile_pool(name="small", bufs=4) as small_pool,
            tc.tile_pool(name="outs", bufs=4) as out_pool,
            tc.tile_pool(name="pbig", bufs=3, space="PSUM") as psum_big,
            tc.tile_pool(name="pctx", bufs=4, space="PSUM") as psum_ctx,
            tc.tile_pool(name="psmall", bufs=1, space="PSUM") as psum_small,
        ):
            # static inputs
            wq_sb = const_pool.tile([128, 8, 128], bf16, tag="wq")
            wk_sb = const_pool.tile([128, 8, 128], bf16, tag="wk")
            wv_sb = const_pool.tile([128, 8, 128], bf16, tag="wv")
            wo_sb = const_pool.tile([128, 8, 128], bf16, tag="wo")
            mask_sb = const_pool.tile([128, 4, QC], bf16, tag="mask")
            id_sb = const_pool.tile([128, 128], bf16, tag="ident")
            nc.sync.dma_start(wq_sb[:], wq_d[:])
            nc.sync.dma_start(wk_sb[:], wk_d[:])
            nc.sync.dma_start(wv_sb[:], wv_d[:])
            nc.sync.dma_start(wo_sb[:], wo_d[:])
            nc.sync.dma_start(mask_sb[:], mask_d[:])
            nc.sync.dma_start(id_sb[:], id_d[:])

            for b in range(B):
                base = b * S
                qT = proj_pool.tile([128, S], bf16, tag="qT")
                kT = proj_pool.tile([128, S], bf16, tag="kT")
                vT = proj_pool.tile([128, S], bf16, tag="vT")

                # ---- projections: qT/kT/vT[:, rows] = W_slice @ x^T ----
                for rc in range(n_rc):
                    g0 = base + rc * RC
                    xt_sb = xt_pool.tile([128, 8, RC], bf16, tag="xt")
                    nc.sync.dma_start(xt_sb[:], xt_d[:, :, g0 : g0 + RC])
                    for w_sb, dst in ((wq_sb, qT), (wk_sb, kT), (wv_sb, vT)):
                        ps = psum_big.tile([128, RC], fp32, tag="pbig")
                        for o in range(8):
                            nc.tensor.matmul(
                                ps[:],
                                w_sb[:, o, :],
                                xt_sb[:, o, :],
                                start=(o == 0),
                                stop=(o == 7),
                            )
                        nc.vector.tensor_copy(dst[:, rc * RC : (rc + 1) * RC], ps[:])

                # ---- V natural layout + ones column, per head ----
                # v_aug[h] is [kpos, 128]: cols 0:64 V_h, col 64 ones, rest zero
                # (padded to 128 weight columns so FWL applies).
                v_aug = [
                    vaug_pool.tile(
                        [128, n_kc, 128], bf16, tag=f"vaug{h}", name=f"vaug{h}_{b}"
                    )
                    for h in range(2)
                ]
                for h in range(2):
                    nc.vector.memset(v_aug[h][:, :, HD + 1 :], 0.0)
                    nc.scalar.activation(
                        v_aug[h][:, :, HD],
                        id_sb[:, 0:n_kc],
                        mybir.ActivationFunctionType.Identity,
                        bias=1.0,
                        scale=0.0,
                    )
                for rk in range(n_kc):
                    pt = psum_small.tile([128, 128], bf16, tag="ptrans")
                    nc.tensor.transpose(pt[:], vT[:, rk * KC : (rk + 1) * KC], id_sb[:])
                    for h in range(2):
                        nc.vector.tensor_copy(
                            v_aug[h][:, rk, 0:HD], pt[:, h * HD : (h + 1) * HD]
                        )

                # ---- causal attention, heads interleaved ----
                ctxT = proj_pool.tile([128, S], bf16, tag="ctxT")
                for qi in range(n_qc):
                    q_sl = slice(qi * QC, (qi + 1) * QC)
                    kc_hi = 4 * qi + 4
                    pcs = [
                        psum_ctx.tile([128, QC], fp32, tag="pctx", name=f"pc{h}_{b}_{qi}")
                        for h in range(2)
                    ]
                    for kc in range(kc_hi):
                        ats = {}
                        for h in range(2):
                            hs = slice(h * HD, (h + 1) * HD)
                            ps = psum_big.tile([128, QC], fp32, tag="pbig")
                            nc.tensor.matmul(
                                ps[:],
                                kT[hs, kc * KC : (kc + 1) * KC],
                                qT[hs, q_sl],
                                start=True,
                                stop=True,
                            )
                            at = attn_pool.tile([128, QC], bf16, tag="at")
                            nc.scalar.activation(
                                at[:],
                                ps[:],
                                mybir.ActivationFunctionType.Exp,
                                scale=0.125,
                            )
                            j = kc - 4 * qi
                            if j >= 0:
                                nc.vector.tensor_mul(at[:], at[:], mask_sb[:, j, :])
                            ats[h] = at
                        for h in range(2):
                            nc.tensor.matmul(
                                pcs[h][:],
                                v_aug[h][:, kc, :],
                                ats[h][:],
                                start=(kc == 0),
                                stop=(kc == kc_hi - 1),
                            )
                    # normalize: one reciprocal for both heads' row-sums
                    # both heads' rowsums packed along free dim of partition 0
                    recs = small_pool.tile([1, 2, QC], fp32, tag="recs")
                    for h in range(2):
                        nc.vector.tensor_copy(
                            recs[0:1, h, :], pcs[h][HD : HD + 1, :]
                        )
                    rrec = small_pool.tile([1, 2, QC], fp32, tag="rrec")
                    nc.vector.reciprocal_approx_fast(rrec[:], recs[:])
                    for h in range(2):
                        hs = slice(h * HD, (h + 1) * HD)
                        rb = small_pool.tile([HD, QC], fp32, tag="rb")
                        nc.gpsimd.partition_broadcast(rb[:], rrec[0:1, h, :])
                        nc.vector.tensor_mul(ctxT[hs, q_sl], pcs[h][0:HD, :], rb[:])

                # ---- partial out-projection (bias added on host) ----
                for mo in range(8):
                    for rc in range(n_rc):
                        r_sl = slice(rc * RC, (rc + 1) * RC)
                        po = psum_big.tile([128, RC], fp32, tag="pbig")
                        nc.tensor.matmul(
                            po[:], wo_sb[:, mo, :], ctxT[:, r_sl], start=True, stop=True
                        )
                        ot = out_pool.tile([128, RC], bf16, tag="ot")
                        nc.vector.tensor_copy(ot[:], po[:])
                        nc.sync.dma_start(
                            out_d[:, mo, base + rc * RC : base + (rc + 1) * RC], ot[:]
                        )

    nc.compile()
    return nc


def _prep_inputs(x, Wq, Wk, Wv, Wo, bo):
    import ml_dtypes

    bf = ml_dtypes.bfloat16

    x = np.ascontiguousarray(np.asarray(x, dtype=np.float32))
    Wq = np.asarray(Wq, dtype=np.float32)
    Wk = np.asarray(Wk, dtype=np.float32)
    Wv = np.asarray(Wv, dtype=np.float32)
    Wo = np.asarray(Wo, dtype=np.float32)

    x_flat = x.reshape(ROWS, D)
    # xt[p, o, n] = x_flat[n, 128*o + p]
    xt = np.ascontiguousarray(
        x_flat.T.reshape(8, 128, ROWS).transpose(1, 0, 2).astype(bf)
    )

    # masks[j][k, q] = 1.0 if k <= q - 128*j
    karr = np.arange(128)[:, None]
    qarr = np.arange(QC)[None, :]
    masks = np.stack(
        [(karr <= qarr - 128 * j).astype(bf) for j in range(4)], axis=1
    )
    masks = np.ascontiguousarray(masks)  # [128, 4, QC]
    ident = np.eye(128, dtype=bf)

    in_maps = []
    for c in range(N_CORES):
        sl = slice(128 * c, 128 * c + 128)

        def wt(W):
            # lhsT chunks: [p(=d within chunk), o(=D chunk), m(=slice feat)]
            Ws = W[sl, :]  # [128, 1024]
            return np.ascontiguousarray(
                Ws.T.reshape(8, 128, 128).transpose(1, 0, 2).astype(bf)
            )

        # wot[f, mo, d] = Wo[128*mo + d, 128*c + f]
        wot = np.ascontiguousarray(
            Wo[:, sl].reshape(8, 128, 128).transpose(2, 0, 1).astype(bf)
        )
        in_maps.append(
            {
                "xt": xt,
                "wqt": wt(Wq),
                "wkt": wt(Wk),
                "wvt": wt(Wv),
                "wot": wot,
                "masks": masks,
                "ident": ident,
            }
        )
    return in_maps


def _run(in_maps, trace=False):
    from concourse.bass_utils import run_bass_kernel_spmd

    if "nc" not in _cache:
        _cache["nc"] = _build()
    return run_bass_kernel_spmd(
        _cache["nc"], in_maps, core_ids=list(range(N_CORES)), trace=trace
    )


def kernel(x, Wq, Wk, Wv, Wo, bo, _trace=False):
    in_maps = _prep_inputs(x, Wq, Wk, Wv, Wo, bo)
    res = _run(in_maps, trace=_trace)
    acc = np.zeros((128, 8, ROWS), dtype=np.float32)
    for r in res.results:
        acc += r["outp"]
    out = acc.transpose(2, 1, 0).reshape(ROWS, D)
    out = out + np.asarray(bo, dtype=np.float32)[None, :]
    out = out.reshape(B, S, D)
    if _trace:
        kernel.last_exec_time_ns = res.exec_time_ns
    return out



# revision 3
# speedup vs baseline: 1.6891x; 1.6891x over previous
"""Trainium2 Bass kernel for 16-head causal MultiHeadAttention.

Problem shapes (hardcoded): x [4, 2048, 1024], Wq/Wk/Wv/Wo [1024, 1024],
bo [1024]. 16 heads, head_dim 64, causal, softmax scale 1/8.

Sharding: batch-major hybrid. Core c owns batch c//2 and head-half c%2
(8 heads = feature slice [512*(c%2), 512*(c%2)+512)). Each core computes
q/k/v for its 8 heads over its batch's 2048 rows, causal attention for
its 8 (batch, head) pairs, and a partial out-projection [1024, 2048].
Host sums the two partials per batch and adds the bias. This cuts
per-core DMA to ~8MB in / 4MB out (vs 16/16 for pure head-parallel).

Device schedule (single pass, no batch loop):
  - q/k projections keep features on partitions (W stationary); v is
    computed directly in natural [row, feat] layout (x stationary) so no
    PE transposes are needed.
  - scores^T [k, q] per head with kT stationary. The contraction dim is
    zero-padded to K=128 (k data for head-pair partner rows is zeroed in
    kT) so every matmul runs in full 128x128 mode - no PE tiling-mode
    switches in the inner loop.
  - causal trimming: for diagonal 128k-blocks only the valid q-columns
    are streamed/exp'd; a single [128,128] lower-tri mask handles the
    leading triangle.
  - softmax: exp on ACT reads both heads' score PSUM banks in ONE
    activation instruction (halves the ~293ns/instr ACT overhead);
    denominators via a ones-column appended to V (M=65 ctx matmuls).
  - out-projection chunks are emitted as soon as the last head-pair
    finishes each 512-row range.
  - projection/out-projection matmul groups are interleaved into the
    attention instruction stream ("background work") so the PE never
    idles while ACT grinds through exps, and HAM stays at K=8/8.

Matmul inputs bf16 (FWL); PSUM accumulation, softmax denominators and
normalization fp32.
"""

import numpy as np

B, S, D, H = 4, 2048, 1024, 16
HD = 64
N_CORES = 8
QC = 512
KC = 128
n_qi = S // QC  # 4 query chunks
n_kc = S // KC  # 16 key chunks (also 128-row chunks for v)
n_rc = 4  # 512-row chunks for projections

_cache = {}


def _build():
    import concourse.bacc as bacc
    import concourse.tile as tile
    from concourse import mybir

    fp32 = mybir.dt.float32
    bf16 = mybir.dt.bfloat16

    nc = bacc.Bacc("TRN2", target_bir_lowering=False)

    xt_d = nc.dram_tensor("xt", [128, 8, S], bf16, kind="ExternalInput")
    wq_d = nc.dram_tensor("wq", [128, 8, 4, 128], bf16, kind="ExternalInput")
    wk_d = nc.dram_tensor("wk", [128, 8, 4, 128], bf16, kind="ExternalInput")
    wv_d = nc.dram_tensor("wv", [128, 8, 512], bf16, kind="ExternalInput")
    wo_d = nc.dram_tensor("wo", [128, 4, 1024], bf16, kind="ExternalInput")
    mask_d = nc.dram_tensor("mask", [128, 2, 128], bf16, kind="ExternalInput")
    out_d = nc.dram_tensor("outp", [128, 8, S], bf16, kind="ExternalOutput")

    with tile.TileContext(nc) as tc:
        with (
            tc.tile_pool(name="const", bufs=1) as cpool,
            tc.tile_pool(name="big", bufs=1) as bigpool,
            tc.tile_pool(name="at", bufs=3) as atpool,
            tc.tile_pool(name="ev", bufs=3) as evpool,
            tc.tile_pool(name="sm", bufs=2) as smpool,
            tc.tile_pool(name="ps", bufs=2, space="PSUM") as ps_pool,
            tc.tile_pool(name="pc", bufs=1, space="PSUM") as pc_pool,
            tc.tile_pool(name="pb", bufs=2, space="PSUM") as pb_pool,
        ):
            # ---- static inputs ----
            wq_sb = cpool.tile([128, 8, 4, 128], bf16, tag="wq")
            wk_sb = cpool.tile([128, 8, 4, 128], bf16, tag="wk")
            wv_sb = cpool.tile([128, 8, 512], bf16, tag="wv")
            wo_sb = cpool.tile([128, 4, 1024], bf16, tag="wo")
            mask_sb = cpool.tile([128, 2, 128], bf16, tag="mask")
            nc.sync.dma_start(wq_sb[:], wq_d[:])
            nc.sync.dma_start(wk_sb[:], wk_d[:])
            nc.sync.dma_start(wv_sb[:], wv_d[:])
            nc.sync.dma_start(wo_sb[:], wo_d[:])
            nc.sync.dma_start(mask_sb[:], mask_d[:])

            # ---- persistent state ----
            xt = bigpool.tile([128, 8, S], bf16, tag="xt")
            qT = bigpool.tile([128, 4, S], bf16, tag="qT")  # [d, pair, n] packed
            kTp = bigpool.tile([128, 4, 2, S], bf16, tag="kTp")  # zero-padded K
            ctxT = bigpool.tile([128, 4, S], bf16, tag="ctxT")
            v_aug = bigpool.tile([128, n_kc, 8, 66], bf16, tag="vaug")

            for rc in range(n_rc):
                sl = slice(rc * QC, (rc + 1) * QC)
                nc.sync.dma_start(xt[:, :, sl], xt_d[:, :, sl])

            # ones column for softmax denominators; zero the partner-head
            # contraction rows of kT so K can be padded to 128
            nc.vector.memset(v_aug[:, :, :, 64:65], 1.0)
            for p in range(4):
                nc.vector.memset(kTp[64:128, p, 0, :], 0.0)
                nc.vector.memset(kTp[0:64, p, 1, :], 0.0)

            # ---- projection work groups ----
            def projqk_group(w_sb, dst_is_q, p, rc):
                sl = slice(rc * QC, (rc + 1) * QC)
                ps = pb_pool.tile([128, QC], fp32, tag="pb", name=f"pqk{p}_{rc}")
                for o in range(8):
                    nc.tensor.matmul(
                        ps[:],
                        w_sb[:, o, p, :],
                        xt[:, o, sl],
                        start=(o == 0),
                        stop=(o == 7),
                    )
                if dst_is_q:
                    nc.vector.tensor_copy(qT[:, p, sl], ps[:])
                else:
                    nc.vector.tensor_copy(kTp[0:64, p, 0, sl], ps[0:64, :])
                    nc.vector.tensor_copy(kTp[64:128, p, 1, sl], ps[64:128, :])

            def projv_group(ci):
                csl = slice(ci * KC, (ci + 1) * KC)
                ps = pb_pool.tile([128, 8, 64], fp32, tag="pb", name=f"pv{ci}")
                for o in range(8):
                    nc.tensor.matmul(
                        ps[:],
                        xt[:, o, csl],
                        wv_sb[:, o, :],
                        start=(o == 0),
                        stop=(o == 7),
                    )
                nc.vector.tensor_copy(v_aug[:, ci, :, 0:64], ps[:])

            def outproj_group(oc, rc):
                sl = slice(rc * QC, (rc + 1) * QC)
                ps = pb_pool.tile([128, QC], fp32, tag="pb", name=f"po{oc}_{rc}")
                for fo in range(4):
                    nc.tensor.matmul(
                        ps[:],
                        wo_sb[:, fo, oc * 128 : (oc + 1) * 128],
                        ctxT[:, fo, sl],
                        start=(fo == 0),
                        stop=(fo == 3),
                    )
                ot = evpool.tile([128, QC], bf16, tag="ot", name=f"ot{oc}_{rc}")
                nc.vector.tensor_copy(ot[:], ps[:])
                nc.sync.dma_start(out_d[:, oc, sl], ot[:])

            # ---- phase A: q/k for pair 0, all of v ----
            for rc in range(n_rc):
                projqk_group(wq_sb, True, 0, rc)
                projqk_group(wk_sb, False, 0, rc)
            for ci in range(4):
                projv_group(ci)

            # ---- background queue (drained inside the attention loop) ----
            bg = []
            for ci in range(4, n_kc):
                bg.append((projv_group, (ci,)))
            for p in range(1, 4):
                for rc in range(n_rc):
                    bg.append((projqk_group, (wq_sb, True, p, rc)))
                    bg.append((projqk_group, (wk_sb, False, p, rc)))

            def drain_bg(n):
                for _ in range(n):
                    if not bg:
                        return
                    f, a = bg.pop(0)
                    f(*a)

            # ---- attention ----
            for p in range(4):
                for qi in range(n_qi):
                    q0 = qi * QC
                    kc_hi = 4 * qi + 4
                    pc0 = pc_pool.tile([128, QC], fp32, tag="pc0", name=f"pc0_{p}_{qi}")
                    pc1 = pc_pool.tile([128, QC], fp32, tag="pc1", name=f"pc1_{p}_{qi}")
                    pcs = (pc0, pc1)
                    for kc in range(kc_hi):
                        j = kc - 4 * qi
                        w0 = 128 * j if j > 0 else 0  # valid q-window start
                        ksl = slice(kc * KC, (kc + 1) * KC)
                        ps = ps_pool.tile(
                            [128, 2, QC], fp32, tag="ps", name=f"ps{p}_{qi}_{kc}"
                        )
                        for h in range(2):
                            nc.tensor.matmul(
                                ps[:, h, w0:QC],
                                kTp[:, p, h, ksl],
                                qT[:, p, q0 + w0 : q0 + QC],
                                start=True,
                                stop=True,
                            )
                        at = atpool.tile(
                            [128, 2, QC], bf16, tag="at", name=f"at{p}_{qi}_{kc}"
                        )
                        nc.scalar.activation(
                            at[:, :, w0:QC],
                            ps[:, :, w0:QC],
                            mybir.ActivationFunctionType.Exp,
                            scale=0.125,
                        )
                        if j >= 0:
                            nc.vector.tensor_mul(
                                at[:, :, w0 : w0 + 128],
                                at[:, :, w0 : w0 + 128],
                                mask_sb[:],
                            )
                        for h in range(2):
                            nc.tensor.matmul(
                                pcs[h][0:65, w0:QC],
                                v_aug[:, kc, 2 * p + h, 0:65],
                                at[:, h, w0:QC],
                                start=(kc == 0),
                                stop=(kc == kc_hi - 1),
                            )
                        if kc % 3 == 2:
                            drain_bg(1)
                    # ---- normalization ----
                    qsl = slice(q0, q0 + QC)
                    rec = smpool.tile([1, 2, QC], fp32, tag="rec", name=f"rc{p}_{qi}")
                    nc.vector.tensor_copy(rec[0:1, 0, :], pc0[64:65, :])
                    nc.vector.tensor_copy(rec[0:1, 1, :], pc1[64:65, :])
                    rrec = smpool.tile([1, 2, QC], fp32, tag="rrec", name=f"rr{p}_{qi}")
                    nc.vector.reciprocal_approx_fast(rrec[:], rec[:])
                    rb = smpool.tile([64, 2, QC], fp32, tag="rb", name=f"rb{p}_{qi}")
                    nc.gpsimd.partition_broadcast(rb[:], rrec[0:1, :, :])
                    nc.vector.tensor_mul(ctxT[0:64, p, qsl], pc0[0:64, :], rb[:, 0, :])
                    nc.vector.tensor_mul(
                        ctxT[64:128, p, qsl], pc1[0:64, :], rb[:, 1, :]
                    )
                    drain_bg(1)
                    if p == 3 and qi < 3:
                        for oc in range(8):
                            bg.append((outproj_group, (oc, qi)))
            # ---- drain remaining background + final out-projection ----
            drain_bg(len(bg))
            for oc in range(8):
                outproj_group(oc, 3)

    nc.compile()
    return nc


def _prep_inputs(x, Wq, Wk, Wv, Wo, bo):
    import ml_dtypes

    bf = ml_dtypes.bfloat16

    x = np.ascontiguousarray(np.asarray(x, dtype=np.float32))
    Wq = np.asarray(Wq, dtype=np.float32)
    Wk = np.asarray(Wk, dtype=np.float32)
    Wv = np.asarray(Wv, dtype=np.float32)
    Wo = np.asarray(Wo, dtype=np.float32)

    karr = np.arange(128)[:, None]
    qarr = np.arange(128)[None, :]
    mask = (karr <= qarr).astype(bf)  # [128 k, 128 q]
    mask2 = np.ascontiguousarray(np.stack([mask, mask], axis=1))  # [128, 2, 128]

    # per batch: xt[p, o, n] = x[b, n, 128o+p]
    xts = []
    for b in range(B):
        xts.append(
            np.ascontiguousarray(
                x[b].T.reshape(8, 128, S).transpose(1, 0, 2).astype(bf)
            )
        )

    in_maps = []
    for c in range(N_CORES):
        b = c // 2
        fh = c % 2
        fsl = slice(fh * 512, fh * 512 + 512)

        def wqk(W):
            # wq[p, o, mo, m] = W[fh*512 + 128*mo + m, 128*o + p]
            Ws = W[fsl, :]  # [512, 1024]
            return np.ascontiguousarray(
                Ws.reshape(4, 128, 8, 128).transpose(3, 2, 0, 1).astype(bf)
            )

        # wv[p, o, f] = Wv[fh*512 + f, 128*o + p]
        wv = np.ascontiguousarray(
            Wv[fsl, :].reshape(512, 8, 128).transpose(2, 1, 0).astype(bf)
        )
        # wo[p, fo, m] = Wo[m, fh*512 + 128*fo + p]
        wo = np.ascontiguousarray(
            Wo[:, fsl].reshape(1024, 4, 128).transpose(2, 1, 0).astype(bf)
        )
        in_maps.append(
            {
                "xt": xts[b],
                "wq": wqk(Wq),
                "wk": wqk(Wk),
                "wv": wv,
                "wo": wo,
                "mask": mask2,
            }
        )
    return in_maps


def _run(in_maps, trace=False):
    from concourse.bass_utils import run_bass_kernel_spmd

    if "nc" not in _cache:
        _cache["nc"] = _build()
    return run_bass_kernel_spmd(
        _cache["nc"], in_maps, core_ids=list(range(N_CORES)), trace=trace
    )


def kernel(x, Wq, Wk, Wv, Wo, bo, _trace=False):
    in_maps = _prep_inputs(x, Wq, Wk, Wv, Wo, bo)
    res = _run(in_maps, trace=_trace)
    bo32 = np.asarray(bo, dtype=np.float32)
    out = np.empty((B, S, D), dtype=np.float32)
    for b in range(B):
        # partial [128, 8, S] -> [1024, S]; out rows = partial.T
        p0 = res.results[2 * b]["outp"].astype(np.float32)
        p1 = res.results[2 * b + 1]["outp"].astype(np.float32)
        acc = (p0 + p1).transpose(1, 0, 2).reshape(D, S)
        out[b] = acc.T + bo32[None, :]
    if _trace:
        kernel.last_exec_time_ns = res.exec_time_ns
    return out
